# revision 29
# baseline (speedup 1.0000x reference)
"""Block-causal sparse attention (QKNorm + RoPE) for Trainium2, 8 NeuronCores.

Sharding: batch*head parallel. 2 batches x 16 heads = 32 (b,h) pairs; core c
handles batch c//4, heads 4*(c%4) .. 4*(c%4)+4. The out-projection runs on
AllGather'd per-head attention within each batch's 4-core group; each core
writes 512 of the 2048 query rows.

v2 (from the 383us baseline):
  - Emission order overlaps the AllGathers with the other query-half's
    attention compute instead of stalling the pipeline on them (the baseline
    lost ~120us to engines idling around the 4 collectives).
  - q/k [token,feat] -> [feat,token] transposes run on the XBAR DMA engine
    (dma_start_transpose) instead of PE transpose + DVE copies; they are
    emitted deferred so they never head-of-line-block a compute queue.
  - k's RMS norm is folded into exp's per-partition scale (1/(8*rms_k) per
    key token): k-hat is never materialized, the normalize multiply covers
    only the q half, and exp needs no max-subtraction since |q-hat|=8 and
    the folded scale bounds |exponent| <= 8.
  - The denominator chain (repack/reciprocal/broadcast) moved off DVE: the
    reciprocal is a Pool-engine divide, the re-broadcast multiply runs on
    Pool, and the DMAs ride the SP hardware-DGE queue.
  - qkv PSUM->SBUF staging goes over DMA instead of the scalar engine (the
    scalar engine is saturated by softmax exp).
  - P2 statistics/rope are split across DVE and Pool; rope cos/sin tables
    are stored once [128, NT, 32] and broadcast via 0-stride views.

All matmuls are bf16 (fp8 was evaluated: every fp8 injection point alone
pushes rel-err past the 2e-2 gate; bf16 sits at ~6e-3). PSUM accumulation
and softmax statistics stay fp32.
"""

import numpy as np

import concourse.bass as bass
from concourse import bacc
import concourse.mybir as mybir
import concourse.tile as tile

F32 = mybir.dt.float32
BF16 = mybir.dt.bfloat16

B, L, D = 2, 2048, 1024
H, DH = 16, 64
NT = L // 128        # 16 token tiles
HPC = 4              # heads per core
N_CORES = 8
GROUPS = [[0, 1, 2, 3], [4, 5, 6, 7]]
ROPE_THETA = 10000.0
EPS = 1e-6

FQK = 512            # q+k feature columns per core
FV = 256
FTOT = 768


def build_program(apply_gamma=False, qkv_bias=False):
    nc = bacc.Bacc(num_devices=N_CORES)

    xT = nc.declare_dram_parameter("xT", [D, L], BF16, isOutput=False)
    gbp = nc.declare_dram_parameter("gb", [1, 1], mybir.dt.uint32, isOutput=False)
    wq = nc.declare_dram_parameter("wq", [D, FTOT], BF16, isOutput=False)
    wo = nc.declare_dram_parameter("wo", [D, D], BF16, isOutput=False)
    cosb = nc.declare_dram_parameter("cosb", [128, NT, 8, 32], BF16, isOutput=False)
    sinb = nc.declare_dram_parameter("sinb", [128, NT, 8, 32], BF16, isOutput=False)
    if apply_gamma:
        # combined per-element gamma (gq*gk), applied to the q half only
        gam = nc.declare_dram_parameter("gam", [4, DH], F32, isOutput=False)
    if qkv_bias:
        bqk = nc.declare_dram_parameter("bqk", [FTOT], F32, isOutput=False)
    # rows [256*half + r] = my shard of query rows [1024*half + 256*rank + r]
    out = nc.declare_dram_parameter("out", [L // 4, D], F32, isOutput=True)

    with tile.TileContext(nc) as tc:
        with (
            tc.tile_pool(name="singles", bufs=1) as singles,
            tc.tile_pool(name="persist", bufs=1) as persist,
            tc.tile_pool(name="dram", bufs=1, space="DRAM") as dram,
        ):
            # V with ones column appended: [tok%128, tile, head, 65]
            vsb = persist.tile([128, NT, HPC, DH + 1], BF16)
            nc.vector.memset(vsb[:, :, :, DH : DH + 1], 1.0)

            # transposed q,k: [feat-of-pair, pr, tok]; pr 0,1 = q pairs,
            # pr 2,3 = k pairs (matches the 3D XBAR transpose row split)
            QKT = persist.tile([128, 4, L], BF16)

            # stats: grp 0..3 = 1/rms_q; grp 4..7 = 1/(8*rms_k) (softmax
            # scale folded in)
            ssqp = persist.tile([128, NT, 8], F32)

            # sqrt bias tiles (eps for the q groups, 64*eps for k groups)
            epst = singles.tile([128, 1], F32)
            nc.gpsimd.memset(epst[:], EPS)
            epskt = singles.tile([128, 1], F32)
            nc.gpsimd.memset(epskt[:], DH * EPS)

            # rope tables, materialized host-side per-group ([0,8]-stride
            # broadcast DMAs explode into 64B descriptors; 0-stride SBUF
            # reads run ~3x slow on DVE)
            cost = singles.tile([128, NT, 8, 32], BF16, name="cos8")
            nc.sync.dma_start(cost[:], cosb[:])
            sint = singles.tile([128, NT, 8, 32], BF16, name="sin8")
            nc.sync.dma_start(sint[:], sinb[:])

            # dedicated prob tile for (half1, j0) with the block-causal
            # exclusion columns pre-zeroed once (no mid-flow memsets)
            pbm = persist.tile([128, 1024], BF16)
            nc.gpsimd.memset(pbm[:, 896:1024], 0.0)

            if apply_gamma:
                gamt = singles.tile([128, 4, DH], F32)
                nc.sync.dma_start(
                    gamt[:],
                    bass.AP(tensor=gam.tensor, offset=gam[:].offset,
                            ap=[[0, 128]] + gam[:].ap))
            if qkv_bias:
                bqkt = singles.tile([128, FTOT], F32)
                nc.sync.dma_start(
                    bqkt[:],
                    bass.AP(tensor=bqk.tensor, offset=bqk[:].offset,
                            ap=[[0, 128]] + bqk[:].ap))

            # qkv weights, split by K-chunk pairs
            wqt = []
            wqr = wq[:].rearrange("(kc p) f -> p kc f", p=128)
            for wi in range(4):
                wt = singles.tile([128, 2, FTOT], BF16, name=f"wqt{wi}")
                nc.sync.dma_start(wt[:], wqr[:, 2 * wi : 2 * wi + 2, :])
                wqt.append(wt)

            wos = persist.tile([128, 8, D], BF16)

            # group-base row of this core's batch within the 8-row A2A
            gbreg = nc.sync.alloc_register("gb_r")
            nc.sync.reg_load(gbreg, gbp[0:1, 0:1])
            gbv = nc.sync.snap(gbreg, donate=True, min_val=0, max_val=4)

            xTr = xT[:].rearrange("(kc p) l -> p kc l", p=128)
            qkrots = {}

            # ---------------- P1 + P2 (per token-quarter) ----------------
            def emit_p12_body(qtr, sbp, psp, xq, late=False):
                t0 = qtr * 4

                qkraw = sbp.tile([128, 4, 8, DH], F32, tag="qkraw",
                                 name=f"qkraw{qtr}")

                for t4 in range(4):
                    t = t0 + t4
                    qk_t = psp.tile([128, FQK], F32, tag="qk_ps",
                                    name=f"qk{qtr}_{t4}")
                    v_t = psp.tile([128, FV], F32, tag="v_ps",
                                   name=f"v{qtr}_{t4}")
                    qk_ps = qk_t[:]
                    v_ps = v_t[:]
                    for kc in range(8):
                        lhsT = xq[:, kc, t4 * 128 : t4 * 128 + 128]
                        wv = wqt[kc // 2][:, kc % 2]
                        nc.tensor.matmul(
                            qk_ps, lhsT, wv[:, 0:FQK],
                            start=(kc == 0), stop=(kc == 7))
                        nc.tensor.matmul(
                            v_ps, lhsT, wv[:, FQK:FTOT],
                            start=(kc == 0), stop=(kc == 7))
                    if qkv_bias:
                        nc.vector.tensor_add(
                            qkraw[:, t4],
                            qk_ps.rearrange("p (g d) -> p g d", d=DH),
                            bqkt[:, 0:FQK].rearrange("p (g d) -> p g d", d=DH))
                        nc.vector.tensor_add(
                            vsb[:, t, :, 0:DH],
                            v_ps.rearrange("p (g d) -> p g d", d=DH),
                            bqkt[:, FQK:FTOT].rearrange("p (g d) -> p g d", d=DH))
                    else:
                        # PSUM->SBUF staging: scalar engine while idle
                        # (quarters 0,1); Pool once exp owns it (2,3)
                        qsrc = qk_ps.rearrange("p (g d) -> p g d", d=DH)
                        vsrc = v_ps.rearrange("p (g d) -> p g d", d=DH)
                        if late:
                            nc.vector.tensor_copy(qkraw[:, t4], qsrc)
                            nc.vector.tensor_copy(vsb[:, t, :, 0:DH], vsrc)
                        else:
                            nc.scalar.copy(qkraw[:, t4], qsrc)
                            nc.scalar.copy(vsb[:, t, :, 0:DH], vsrc)

                # RMS statistics (square on the scalar engine: it is idle
                # during this phase, DVE is the pacer)
                sqt = sbp.tile([128, 4, 8, DH], BF16, tag="sqt",
                               name=f"sqt{qtr}")
                nc.scalar.activation(sqt[:], qkraw[:],
                                     mybir.ActivationFunctionType.Square)
                ssq = ssqp[:, t0 : t0 + 4, :]
                nc.vector.reduce_sum(ssq, sqt[:], axis=mybir.AxisListType.X)
                nc.scalar.activation(
                    ssqp[:, t0 : t0 + 4, 0:4], ssqp[:, t0 : t0 + 4, 0:4],
                    mybir.ActivationFunctionType.Sqrt,
                    bias=epst[:], scale=1.0 / DH)
                nc.scalar.activation(
                    ssqp[:, t0 : t0 + 4, 4:8], ssqp[:, t0 : t0 + 4, 4:8],
                    mybir.ActivationFunctionType.Sqrt,
                    bias=epskt[:], scale=1.0)
                nc.vector.reciprocal(ssq, ssq)

                # RoPE
                cqv = cost[:, t0 : t0 + 4, :, :]
                sqv = sint[:, t0 : t0 + 4, :, :]
                qkrot = sbp.tile([128, 4, 8, DH], BF16, tag="qkrot",
                                 name=f"qkrot{qtr}")
                qkrots[qtr] = qkrot
                q1 = qkraw[:, :, :, 0:32]
                q2 = qkraw[:, :, :, 32:64]
                mA = sbp.tile([128, 4, 8, 32], BF16, tag="mA", name=f"mA{qtr}")
                mB = sbp.tile([128, 4, 8, 32], BF16, tag="mB", name=f"mB{qtr}")
                nc.vector.tensor_mul(mA[:], q1, cqv)
                nc.vector.tensor_mul(mB[:], q2, sqv)
                nc.gpsimd.tensor_sub(qkrot[:, :, :, 0:32], mA[:], mB[:])
                mC = sbp.tile([128, 4, 8, 32], BF16, tag="mA", name=f"mC{qtr}")
                mD = sbp.tile([128, 4, 8, 32], BF16, tag="mB", name=f"mD{qtr}")
                nc.gpsimd.tensor_mul(mC[:], q2, cqv)
                nc.gpsimd.tensor_mul(mD[:], q1, sqv)
                nc.gpsimd.tensor_add(qkrot[:, :, :, 32:64], mC[:], mD[:])

                # apply 1/rms to the q half only (k's is folded into exp)
                rq = ssqp[:, t0 : t0 + 4, 0:4]
                rqv = bass.AP(tensor=rq.tensor, offset=rq.offset,
                              ap=rq.ap + [[0, DH]])
                nc.vector.tensor_mul(qkrot[:, :, 0:4, :],
                                     qkrot[:, :, 0:4, :], rqv)
                if apply_gamma:
                    gview = bass.AP(
                        tensor=gamt.tensor, offset=gamt[:].offset,
                        ap=[gamt[:].ap[0], [0, 4]] + gamt[:].ap[1:])
                    nc.vector.tensor_mul(qkrot[:, :, 0:4, :],
                                         qkrot[:, :, 0:4, :], gview)

            def emit_p12_tps(qtr, eng):
                # XBAR DMA transpose, 3D out: in [128 tok, 512 feat] ->
                # out[f, pr, tok] = in[tok, pr*128 + f]
                t0 = qtr * 4
                qkrot = qkrots[qtr]
                for t4 in range(4):
                    t = t0 + t4
                    eng.dma_start_transpose(
                        QKT[:, :, t * 128 : (t + 1) * 128],
                        qkrot[:, t4, :, :])

            # ---------------- P3: one head of one query half ----------------
            def emit_p3_head(half, h, scps, atps, pbsb, recp,
                             early=False):
                qlo = half * 1024
                jmax = 8 if half == 0 else 16
                pb0 = (h % 2) * 64
                at = atps.tile([DH + 1, 1024], F32, tag="at",
                               name=f"at{half}_{h}")
                for j in range(jmax):
                    wlo = max(j * 128, qlo)     # global query col start
                    w = qlo + 1024 - wlo
                    sc = scps.tile([128, 1024], F32, tag="sc",
                                   name=f"sc{half}_{h}_{j}")
                    for c0 in range(0, w, 512):
                        cw = min(512, w - c0)
                        nc.tensor.matmul(
                            sc[:, c0 : c0 + cw],
                            QKT[pb0 : pb0 + 64, 2 + h // 2,
                                j * 128 : (j + 1) * 128],
                            QKT[pb0 : pb0 + 64, h // 2,
                                wlo + c0 : wlo + c0 + cw],
                            start=True, stop=True)
                    if half == 1 and j == 0:
                        # mask tile: cols 896.. pre-zeroed (last query frame
                        # can't see key frame 0); exp writes only 0..896
                        pb = pbm
                        wexp = 896
                    else:
                        pb = pbsb.tile([128, 1024], BF16, tag="pb",
                                       name=f"pb{half}_{h}_{j}")
                        wexp = w
                    # exp with k's 1/(8*rms) folded in as per-key scale
                    rk = ssqp[:, j, 4 + h : 5 + h]
                    nc.scalar.activation(
                        pb[:, 0:wexp], sc[:, 0:wexp],
                        mybir.ActivationFunctionType.Exp, scale=rk)
                    s_rel = wlo - qlo           # window start within half
                    for b0 in range(0, 1024, 512):
                        seg0 = max(s_rel, b0)
                        seg1 = b0 + 512
                        if seg0 >= seg1:
                            continue
                        nc.tensor.matmul(
                            at[:, seg0:seg1],
                            vsb[:, j, h, :],
                            pb[:, seg0 - s_rel : seg1 - s_rel],
                            start=(j == 0),
                            stop=(j == jmax - 1
                                  or (j + 1) * 128 >= qlo + seg1))
                # stage attn rows + the denominator row together, then
                # scatter into the AllToAll input (q-sharded per dest core);
                # normalization happens post-exchange on the P4 side
                st = recp.tile([DH + 1, 1024], BF16, tag="st65",
                               name=f"st{half}_{h}")

                def finish():
                    nc.vector.tensor_copy(st[:], at[:])
                    stv = st[:].rearrange("r (g c) -> r g c", g=4)
                    for b in range(2):
                        dst = a2a_in[half][4 * b : 4 * b + 4,
                                           65 * h : 65 * h + 65, :]
                        nc.sync.dma_start(dst.rearrange("g r c -> r g c"),
                                          stv)
                # early heads run concurrently with quarters 2,3: the
                # PV-dependent staging copy is deferred so DVE's queue
                # keeps flowing on P2 work
                if early:
                    return finish
                finish()

            # ------------- AllToAll exchange (one per half) -------------
            # 8-core mesh AllToAll (4-core groups are not mesh-supported).
            # in[g] = my 4 heads' (attn+den) rows for query columns
            # 256*(g%4); receivers ignore cross-batch blocks. out[s] = core
            # s's heads for my query shard; P4 reads rows [gb, gb+4).
            a2a_in = [dram.tile([8, 4 * 65, 256], BF16, name=f"a2ain{hf}")
                      for hf in range(2)]
            a2a_out = [dram.tile([8, 4 * 65, 256], BF16, name=f"a2aout{hf}")
                       for hf in range(2)]

            def emit_a2a(half):
                nc.gpsimd.collective_compute(
                    "AllToAll", mybir.AluOpType.bypass,
                    replica_groups=[list(range(8))],
                    ins=[a2a_in[half][:].opt()],
                    outs=[a2a_out[half][:].opt()])

            def emit_p4(half, agp, osb, scps):
                # a2a_out[half][s, 65*h + r, q'] = source core s's head h:
                # rows 0..63 unnormalized attn channels, row 64 the denom
                ao = a2a_out[half]
                agsb = []
                for pr in range(2):
                    a = agp.tile([128, 4, 256], BF16, tag=f"agsb{pr}",
                                 name=f"agsb{half}_{pr}")
                    dn = agp.tile([128, 4, 256], BF16, tag=f"denA{pr}",
                                  name=f"denA{half}_{pr}")
                    for lh in range(2):
                        h = 2 * pr + lh
                        av = ao[bass.ds(gbv, 4), 65 * h : 65 * h + 64, :]
                        nc.sync.dma_start(
                            a[lh * 64 : lh * 64 + 64],
                            av.rearrange("s p q -> p s q"))
                        dview = ao[bass.ds(gbv, 4), 65 * h + 64, :]
                        nc.sync.dma_start(
                            dn[lh * 64 : lh * 64 + 64],
                            bass.AP(tensor=dview.tensor, offset=dview.offset,
                                    ap=[[0, 64]] + dview.ap))
                    rdn = agp.tile([128, 4, 256], F32, tag=f"rdenA{pr}",
                                   name=f"rdenA{half}_{pr}")
                    nc.vector.reciprocal(rdn[:], dn[:])
                    nc.gpsimd.tensor_mul(a[:], a[:], rdn[:])
                    agsb.append(a)
                for qt in range(2):
                    ost = osb.tile([128, D], F32, tag="ost",
                                   name=f"ost{half}_{qt}")
                    for o0 in range(0, D, 512):
                        op = scps.tile([128, 512], F32, tag="sc",
                                       name=f"op{half}_{qt}_{o0}")
                        for pr in range(2):
                            for s in range(4):
                                nc.tensor.matmul(
                                    op[:],
                                    agsb[pr][:, s, qt * 128 : (qt + 1) * 128],
                                    wos[:, s * 2 + pr, o0 : o0 + 512],
                                    start=(pr == 0 and s == 0),
                                    stop=(pr == 1 and s == 3))
                        nc.vector.tensor_copy(ost[:, o0 : o0 + 512], op[:])
                    nc.sync.dma_start(
                        out[half * 256 + qt * 128
                            : half * 256 + (qt + 1) * 128, :],
                        ost[:])

            # ---------------- emission schedule ----------------
            # Phase 1: all four token-quarters (PE-bound on the qkv
            # projection; DVE/Pool run rope/stats; XBAR transposes trail
            # each quarter). Phase 2: half-0 attention, with its two
            # pair-AllGathers overlapping half-1 attention. Phase 3: half-1
            # attention with per-head AllGathers (short tail) and the
            # out-projections.
            xqs = []
            for qtr in range(4):
                xqt = persist.tile([128, 8, 512], BF16, name=f"xq{qtr}")
                nc.sync.dma_start(
                    xqt[:], xTr[:, :, qtr * 512 : qtr * 512 + 512])
                xqs.append(xqt)

            with (
                tc.tile_pool(name="p12sbA", bufs=2) as p12sbA,
                tc.tile_pool(name="p12psA", bufs=2, space="PSUM") as p12psA,
            ):
                emit_p12_body(0, p12sbA, p12psA, xqs[0])
                emit_p12_tps(0, nc.sync)
                emit_p12_body(1, p12sbA, p12psA, xqs[1])
                emit_p12_tps(1, nc.sync)

            with (
                tc.tile_pool(name="scps", bufs=2, space="PSUM") as scps,
                tc.tile_pool(name="atps", bufs=1, space="PSUM") as atps,
                tc.tile_pool(name="pbsb", bufs=3) as pbsb,
                tc.tile_pool(name="recp", bufs=2) as recp,
                tc.tile_pool(name="osb", bufs=2) as osb,
                tc.tile_pool(name="agp", bufs=2) as agp,
            ):
                with (
                    tc.tile_pool(name="p12sbB", bufs=2) as p12sbB,
                    tc.tile_pool(name="p12psB", bufs=1, space="PSUM") as p12psB,
                ):
                    fin0 = emit_p3_head(0, 0, scps, atps, pbsb, recp,
                                        early=True)
                    emit_p12_body(2, p12sbB, p12psB, xqs[2], late=True)
                    emit_p12_tps(2, nc.sync)
                    fin0()
                    fin1 = emit_p3_head(0, 1, scps, atps, pbsb, recp,
                                        early=True)
                    emit_p12_body(3, p12sbB, p12psB, xqs[3], late=True)
                    emit_p12_tps(3, nc.sync)
                    fin1()
                emit_p3_head(0, 2, scps, atps, pbsb, recp)
                nc.sync.dma_start(
                    wos[:], wo[:].rearrange("(kc p) o -> p kc o", p=128))
                emit_p3_head(0, 3, scps, atps, pbsb, recp)
                emit_a2a(0)
                for h in range(4):
                    emit_p3_head(1, h, scps, atps, pbsb, recp)
                emit_a2a(1)
                emit_p4(0, agp, osb, scps)
                emit_p4(1, agp, osb, scps)

    nc.compile()
    return nc


_PROG_CACHE = {}


def _get_program(key):
    if key not in _PROG_CACHE:
        _PROG_CACHE[key] = build_program(*key)
    return _PROG_CACHE[key]


def _host_inputs(x, W_qkv, b_qkv, W_out, b_out, q_gamma, k_gamma):
    import ml_dtypes
    mmnp = ml_dtypes.bfloat16
    x = np.asarray(x, dtype=np.float32)
    W_qkv = np.asarray(W_qkv, dtype=np.float32)
    b_qkv = np.asarray(b_qkv, dtype=np.float32)
    W_out = np.asarray(W_out, dtype=np.float32)
    q_gamma = np.asarray(q_gamma, dtype=np.float32)
    k_gamma = np.asarray(k_gamma, dtype=np.float32)

    apply_gamma = not (np.all(q_gamma == 1.0) and np.all(k_gamma == 1.0))
    qkv_bias = bool(np.any(b_qkv))

    # rope tables: pos = t*128 + p
    pos = np.arange(L, dtype=np.float64).reshape(NT, 128).T  # [128, NT]
    inv = 1.0 / (ROPE_THETA ** (np.arange(32, dtype=np.float64) / 32.0))
    ang = pos[:, :, None] * inv[None, None, :]               # [128, NT, 32]
    cosb = np.ascontiguousarray(np.broadcast_to(
        np.cos(ang)[:, :, None, :], (128, NT, 8, 32))).astype(mmnp)
    sinb = np.ascontiguousarray(np.broadcast_to(
        np.sin(ang)[:, :, None, :], (128, NT, 8, 32))).astype(mmnp)

    Wq = W_qkv[0 * D : 1 * D]
    Wk = W_qkv[1 * D : 2 * D]
    Wv = W_qkv[2 * D : 3 * D]
    WoT = np.ascontiguousarray(W_out.T)  # [d_in, d_out]

    in_maps = []
    for c in range(N_CORES):
        b = c // 4
        h0 = 4 * (c % 4)
        rows = slice(h0 * DH, (h0 + HPC) * DH)
        wq_c = np.ascontiguousarray(
            np.concatenate([Wq[rows], Wk[rows], Wv[rows]], axis=0).T)
        m = {
            "xT": np.ascontiguousarray(x[b].T).astype(mmnp),
            "wq": wq_c.astype(mmnp),
            "wo": WoT.astype(mmnp),
            "cosb": cosb,
            "sinb": sinb,
            "gb": np.array([[4 * (c // 4)]], dtype=np.uint32),
        }
        if apply_gamma:
            m["gam"] = np.ascontiguousarray(
                np.broadcast_to(q_gamma * k_gamma, (4, DH)).astype(np.float32))
        if qkv_bias:
            m["bqk"] = np.ascontiguousarray(np.concatenate(
                [b_qkv[0 * D : 1 * D][rows], b_qkv[1 * D : 2 * D][rows],
                 b_qkv[2 * D : 3 * D][rows]]))
        in_maps.append(m)

    key = (apply_gamma, qkv_bias)
    return key, in_maps


def _assemble(results, b_out):
    y = np.empty((B, L, D), dtype=np.float32)
    for c in range(N_CORES):
        b = c // 4
        r = c % 4
        o = results[c]["out"]
        for half in range(2):
            rows = slice(1024 * half + 256 * r, 1024 * half + 256 * r + 256)
            y[b, rows, :] = o[256 * half : 256 * half + 256]
    b_out = np.asarray(b_out, dtype=np.float32)
    if np.any(b_out):
        y += b_out
    return y


def _install_ntff_hook():
    """Register the axon NTFF profiling hook (the container's antenv stub
    lacks axon_hooks; replicate what trn_boot would have registered)."""
    import sys
    import types
    try:
        from antenv.axon_hooks import get_axon_ntff_profile_hook  # noqa: F401
        return
    except ImportError:
        pass
    try:
        from trn_agent_boot.trn_boot import _ntff_profile_via_ctypes
        hook = _ntff_profile_via_ctypes("/opt/axon/libaxon_pjrt.so")
    except Exception:
        hook = None
    import antenv
    mod = types.ModuleType("antenv.axon_hooks")
    mod.get_axon_ntff_profile_hook = lambda: hook
    mod.set_axon_ntff_profile_hook = lambda h: None
    antenv.axon_hooks = mod
    sys.modules["antenv.axon_hooks"] = mod


def kernel(x, W_qkv, b_qkv, W_out, b_out, q_gamma, k_gamma, _trace=False):
    from concourse.bass_utils import run_bass_kernel_spmd
    if _trace:
        _install_ntff_hook()

    key, in_maps = _host_inputs(x, W_qkv, b_qkv, W_out, b_out,
                                q_gamma, k_gamma)
    nc = _get_program(key)
    res = run_bass_kernel_spmd(nc, in_maps, core_ids=list(range(N_CORES)),
                               trace=_trace,
                               trace_cores=list(range(N_CORES)) if _trace else None)
    y = _assemble(res.results, b_out)
    if _trace:
        return y, res
    return y


# revision 32
# speedup vs baseline: 1.0246x; 1.0246x over previous
"""Block-causal sparse attention (QKNorm + RoPE) for Trainium2, 8 NeuronCores.

Sharding: batch*head parallel. 2 batches x 16 heads = 32 (b,h) pairs; core c
handles batch c//4, heads 4*(c%4) .. 4*(c%4)+4. The out-projection runs on
AllGather'd per-head attention within each batch's 4-core group; each core
writes 512 of the 2048 query rows.

v2 (from the 383us baseline):
  - Emission order overlaps the AllGathers with the other query-half's
    attention compute instead of stalling the pipeline on them (the baseline
    lost ~120us to engines idling around the 4 collectives).
  - q/k [token,feat] -> [feat,token] transposes run on the XBAR DMA engine
    (dma_start_transpose) instead of PE transpose + DVE copies; they are
    emitted deferred so they never head-of-line-block a compute queue.
  - k's RMS norm is folded into exp's per-partition scale (1/(8*rms_k) per
    key token): k-hat is never materialized, the normalize multiply covers
    only the q half, and exp needs no max-subtraction since |q-hat|=8 and
    the folded scale bounds |exponent| <= 8.
  - The denominator chain (repack/reciprocal/broadcast) moved off DVE: the
    reciprocal is a Pool-engine divide, the re-broadcast multiply runs on
    Pool, and the DMAs ride the SP hardware-DGE queue.
  - qkv PSUM->SBUF staging goes over DMA instead of the scalar engine (the
    scalar engine is saturated by softmax exp).
  - P2 statistics/rope are split across DVE and Pool; rope cos/sin tables
    are stored once [128, NT, 32] and broadcast via 0-stride views.

All matmuls are bf16 (fp8 was evaluated: every fp8 injection point alone
pushes rel-err past the 2e-2 gate; bf16 sits at ~6e-3). PSUM accumulation
and softmax statistics stay fp32.
"""

import numpy as np

import concourse.bass as bass
from concourse import bacc
import concourse.mybir as mybir
import concourse.tile as tile

F32 = mybir.dt.float32
BF16 = mybir.dt.bfloat16

B, L, D = 2, 2048, 1024
H, DH = 16, 64
NT = L // 128        # 16 token tiles
HPC = 4              # heads per core
N_CORES = 8
GROUPS = [[0, 1, 2, 3], [4, 5, 6, 7]]
ROPE_THETA = 10000.0
EPS = 1e-6

FQK = 512            # q+k feature columns per core
FV = 256
FTOT = 768


def build_program(apply_gamma=False, qkv_bias=False):
    nc = bacc.Bacc(num_devices=N_CORES)

    xT = nc.declare_dram_parameter("xT", [D, L], BF16, isOutput=False)
    gbp = nc.declare_dram_parameter("gb", [1, 1], mybir.dt.uint32, isOutput=False)
    wq = nc.declare_dram_parameter("wq", [D, FTOT], BF16, isOutput=False)
    wo = nc.declare_dram_parameter("wo", [D, D], BF16, isOutput=False)
    cosb = nc.declare_dram_parameter("cosb", [128, NT, 8, 32], BF16, isOutput=False)
    sinb = nc.declare_dram_parameter("sinb", [128, NT, 8, 32], BF16, isOutput=False)
    if apply_gamma:
        # combined per-element gamma (gq*gk), applied to the q half only
        gam = nc.declare_dram_parameter("gam", [4, DH], F32, isOutput=False)
    if qkv_bias:
        bqk = nc.declare_dram_parameter("bqk", [FTOT], F32, isOutput=False)
    # rows [256*half + r] = my shard of query rows [1024*half + 256*rank + r]
    out = nc.declare_dram_parameter("out", [L // 4, D], F32, isOutput=True)

    with tile.TileContext(nc) as tc:
        with (
            tc.tile_pool(name="singles", bufs=1) as singles,
            tc.tile_pool(name="persist", bufs=1) as persist,
            tc.tile_pool(name="dram", bufs=1, space="DRAM") as dram,
        ):
            # V with ones column appended: [tok%128, tile, head, 65]
            vsb = persist.tile([128, NT, HPC, DH + 1], BF16)
            nc.vector.memset(vsb[:, :, :, DH : DH + 1], 1.0)

            # transposed q,k: [feat-of-pair, pr, tok]; pr 0,1 = q pairs,
            # pr 2,3 = k pairs (matches the 3D XBAR transpose row split)
            QKT = persist.tile([128, 4, L], BF16)

            # stats: grp 0..3 = 1/rms_q; grp 4..7 = 1/(8*rms_k) (softmax
            # scale folded in)
            ssqp = persist.tile([128, NT, 8], F32)

            # sqrt bias tiles (eps for the q groups, 64*eps for k groups)
            epst = singles.tile([128, 1], F32)
            nc.gpsimd.memset(epst[:], EPS)
            epskt = singles.tile([128, 1], F32)
            nc.gpsimd.memset(epskt[:], DH * EPS)

            # rope tables, materialized host-side per-group ([0,8]-stride
            # broadcast DMAs explode into 64B descriptors; 0-stride SBUF
            # reads run ~3x slow on DVE)
            cost = singles.tile([128, NT, 8, 32], BF16, name="cos8")
            nc.sync.dma_start(cost[:], cosb[:])
            sint = singles.tile([128, NT, 8, 32], BF16, name="sin8")
            nc.sync.dma_start(sint[:], sinb[:])

            # dedicated prob tile for (half1, j0) with the block-causal
            # exclusion columns pre-zeroed once (no mid-flow memsets)
            pbm = persist.tile([128, 1024], BF16)
            nc.gpsimd.memset(pbm[:, 896:1024], 0.0)

            if apply_gamma:
                gamt = singles.tile([128, 4, DH], F32)
                nc.sync.dma_start(
                    gamt[:],
                    bass.AP(tensor=gam.tensor, offset=gam[:].offset,
                            ap=[[0, 128]] + gam[:].ap))
            if qkv_bias:
                bqkt = singles.tile([128, FTOT], F32)
                nc.sync.dma_start(
                    bqkt[:],
                    bass.AP(tensor=bqk.tensor, offset=bqk[:].offset,
                            ap=[[0, 128]] + bqk[:].ap))

            # qkv weights, split by K-chunk pairs
            wqt = []
            wqr = wq[:].rearrange("(kc p) f -> p kc f", p=128)
            for wi in range(4):
                wt = singles.tile([128, 2, FTOT], BF16, name=f"wqt{wi}")
                nc.sync.dma_start(wt[:], wqr[:, 2 * wi : 2 * wi + 2, :])
                wqt.append(wt)

            wos = persist.tile([128, 8, D], BF16)

            # group-base row of this core's batch within the 8-row A2A
            gbreg = nc.sync.alloc_register("gb_r")
            nc.sync.reg_load(gbreg, gbp[0:1, 0:1])
            gbv = nc.sync.snap(gbreg, donate=True, min_val=0, max_val=4)

            xTr = xT[:].rearrange("(kc p) l -> p kc l", p=128)
            qkrots = {}

            # ---------------- P1 + P2 (per token-quarter) ----------------
            def emit_p12_body(qtr, sbp, psp, xq):
                t0 = qtr * 4

                qkraw = sbp.tile([128, 4, 8, DH], F32, tag="qkraw",
                                 name=f"qkraw{qtr}")

                for t4 in range(4):
                    t = t0 + t4
                    qk_t = psp.tile([128, FQK], F32, tag="qk_ps",
                                    name=f"qk{qtr}_{t4}")
                    v_t = psp.tile([128, FV], F32, tag="v_ps",
                                   name=f"v{qtr}_{t4}")
                    qk_ps = qk_t[:]
                    v_ps = v_t[:]
                    for kc in range(8):
                        lhsT = xq[:, kc, t4 * 128 : t4 * 128 + 128]
                        wv = wqt[kc // 2][:, kc % 2]
                        nc.tensor.matmul(
                            qk_ps, lhsT, wv[:, 0:FQK],
                            start=(kc == 0), stop=(kc == 7))
                        nc.tensor.matmul(
                            v_ps, lhsT, wv[:, FQK:FTOT],
                            start=(kc == 0), stop=(kc == 7))
                    if qkv_bias:
                        nc.vector.tensor_add(
                            qkraw[:, t4],
                            qk_ps.rearrange("p (g d) -> p g d", d=DH),
                            bqkt[:, 0:FQK].rearrange("p (g d) -> p g d", d=DH))
                        nc.vector.tensor_add(
                            vsb[:, t, :, 0:DH],
                            v_ps.rearrange("p (g d) -> p g d", d=DH),
                            bqkt[:, FQK:FTOT].rearrange("p (g d) -> p g d", d=DH))
                    else:
                        # PSUM->SBUF staging on the scalar engine (idle
                        # during the projection phase)
                        nc.scalar.copy(
                            qkraw[:, t4],
                            qk_ps.rearrange("p (g d) -> p g d", d=DH))
                        nc.scalar.copy(
                            vsb[:, t, :, 0:DH],
                            v_ps.rearrange("p (g d) -> p g d", d=DH))

                # RMS statistics (square on the scalar engine: it is idle
                # during this phase, DVE is the pacer)
                sqt = sbp.tile([128, 4, 8, DH], BF16, tag="sqt",
                               name=f"sqt{qtr}")
                nc.scalar.activation(sqt[:], qkraw[:],
                                     mybir.ActivationFunctionType.Square)
                ssq = ssqp[:, t0 : t0 + 4, :]
                nc.vector.reduce_sum(ssq, sqt[:], axis=mybir.AxisListType.X)
                nc.scalar.activation(
                    ssqp[:, t0 : t0 + 4, 0:4], ssqp[:, t0 : t0 + 4, 0:4],
                    mybir.ActivationFunctionType.Sqrt,
                    bias=epst[:], scale=1.0 / DH)
                nc.scalar.activation(
                    ssqp[:, t0 : t0 + 4, 4:8], ssqp[:, t0 : t0 + 4, 4:8],
                    mybir.ActivationFunctionType.Sqrt,
                    bias=epskt[:], scale=1.0)
                nc.vector.reciprocal(ssq, ssq)

                # RoPE
                cqv = cost[:, t0 : t0 + 4, :, :]
                sqv = sint[:, t0 : t0 + 4, :, :]
                qkrot = sbp.tile([128, 4, 8, DH], BF16, tag="qkrot",
                                 name=f"qkrot{qtr}")
                qkrots[qtr] = qkrot
                q1 = qkraw[:, :, :, 0:32]
                q2 = qkraw[:, :, :, 32:64]
                mA = sbp.tile([128, 4, 8, 32], BF16, tag="mA", name=f"mA{qtr}")
                mB = sbp.tile([128, 4, 8, 32], BF16, tag="mB", name=f"mB{qtr}")
                nc.vector.tensor_mul(mA[:], q1, cqv)
                nc.vector.tensor_mul(mB[:], q2, sqv)
                nc.gpsimd.tensor_sub(qkrot[:, :, :, 0:32], mA[:], mB[:])
                mC = sbp.tile([128, 4, 8, 32], BF16, tag="mA", name=f"mC{qtr}")
                mD = sbp.tile([128, 4, 8, 32], BF16, tag="mB", name=f"mD{qtr}")
                nc.gpsimd.tensor_mul(mC[:], q2, cqv)
                nc.gpsimd.tensor_mul(mD[:], q1, sqv)
                nc.gpsimd.tensor_add(qkrot[:, :, :, 32:64], mC[:], mD[:])

                # apply 1/rms to the q half only (k's is folded into exp)
                rq = ssqp[:, t0 : t0 + 4, 0:4]
                rqv = bass.AP(tensor=rq.tensor, offset=rq.offset,
                              ap=rq.ap + [[0, DH]])
                nc.vector.tensor_mul(qkrot[:, :, 0:4, :],
                                     qkrot[:, :, 0:4, :], rqv)
                if apply_gamma:
                    gview = bass.AP(
                        tensor=gamt.tensor, offset=gamt[:].offset,
                        ap=[gamt[:].ap[0], [0, 4]] + gamt[:].ap[1:])
                    nc.vector.tensor_mul(qkrot[:, :, 0:4, :],
                                         qkrot[:, :, 0:4, :], gview)

            def emit_p12_tps(qtr, eng):
                # XBAR DMA transpose, 3D out: in [128 tok, 512 feat] ->
                # out[f, pr, tok] = in[tok, pr*128 + f]
                t0 = qtr * 4
                qkrot = qkrots[qtr]
                for t4 in range(4):
                    t = t0 + t4
                    eng.dma_start_transpose(
                        QKT[:, :, t * 128 : (t + 1) * 128],
                        qkrot[:, t4, :, :])

            # ---------------- P3: one head of one query half ----------------
            def emit_p3_head(half, h, scps, atps, pbsb, recp):
                qlo = half * 1024
                jmax = 8 if half == 0 else 16
                pb0 = (h % 2) * 64
                at = atps.tile([DH + 1, 1024], F32, tag="at",
                               name=f"at{half}_{h}")
                for j in range(jmax):
                    wlo = max(j * 128, qlo)     # global query col start
                    w = qlo + 1024 - wlo
                    sc = scps.tile([128, 1024], F32, tag="sc",
                                   name=f"sc{half}_{h}_{j}")
                    for c0 in range(0, w, 512):
                        cw = min(512, w - c0)
                        nc.tensor.matmul(
                            sc[:, c0 : c0 + cw],
                            QKT[pb0 : pb0 + 64, 2 + h // 2,
                                j * 128 : (j + 1) * 128],
                            QKT[pb0 : pb0 + 64, h // 2,
                                wlo + c0 : wlo + c0 + cw],
                            start=True, stop=True)
                    if half == 1 and j == 0:
                        # mask tile: cols 896.. pre-zeroed (last query frame
                        # can't see key frame 0); exp writes only 0..896
                        pb = pbm
                        wexp = 896
                    else:
                        pb = pbsb.tile([128, 1024], BF16, tag="pb",
                                       name=f"pb{half}_{h}_{j}")
                        wexp = w
                    # exp with k's 1/(8*rms) folded in as per-key scale
                    rk = ssqp[:, j, 4 + h : 5 + h]
                    nc.scalar.activation(
                        pb[:, 0:wexp], sc[:, 0:wexp],
                        mybir.ActivationFunctionType.Exp, scale=rk)
                    s_rel = wlo - qlo           # window start within half
                    for b0 in range(0, 1024, 512):
                        seg0 = max(s_rel, b0)
                        seg1 = b0 + 512
                        if seg0 >= seg1:
                            continue
                        nc.tensor.matmul(
                            at[:, seg0:seg1],
                            vsb[:, j, h, :],
                            pb[:, seg0 - s_rel : seg1 - s_rel],
                            start=(j == 0),
                            stop=(j == jmax - 1
                                  or (j + 1) * 128 >= qlo + seg1))
                # stage attn rows + the denominator row together, then
                # scatter into the AllToAll input (q-sharded per dest core);
                # normalization happens post-exchange on the P4 side
                st = recp.tile([DH + 1, 1024], BF16, tag="st65",
                               name=f"st{half}_{h}")
                nc.vector.tensor_copy(st[:], at[:])
                stv = st[:].rearrange("r (g c) -> r g c", g=4)
                for b in range(2):
                    dst = a2a_in[half][4 * b : 4 * b + 4,
                                       65 * h : 65 * h + 65, :]
                    nc.sync.dma_start(dst.rearrange("g r c -> r g c"), stv)

            # ------------- AllToAll exchange (one per half) -------------
            # 8-core mesh AllToAll (4-core groups are not mesh-supported).
            # in[g] = my 4 heads' (attn+den) rows for query columns
            # 256*(g%4); receivers ignore cross-batch blocks. out[s] = core
            # s's heads for my query shard; P4 reads rows [gb, gb+4).
            a2a_in = [dram.tile([8, 4 * 65, 256], BF16, name=f"a2ain{hf}")
                      for hf in range(2)]
            a2a_out = [dram.tile([8, 4 * 65, 256], BF16, name=f"a2aout{hf}")
                       for hf in range(2)]

            def emit_a2a(half):
                nc.gpsimd.collective_compute(
                    "AllToAll", mybir.AluOpType.bypass,
                    replica_groups=[list(range(8))],
                    ins=[a2a_in[half][:].opt()],
                    outs=[a2a_out[half][:].opt()])

            def emit_p4(half, agp, osb, scps):
                # a2a_out[half][s, 65*h + r, q'] = source core s's head h:
                # rows 0..63 unnormalized attn channels, row 64 the denom
                ao = a2a_out[half]
                agsb = []
                tiles = []
                for pr in range(2):
                    a = agp.tile([128, 4, 256], BF16, tag=f"agsb{pr}",
                                 name=f"agsb{half}_{pr}")
                    dn = agp.tile([128, 4, 256], BF16, tag=f"denA{pr}",
                                  name=f"denA{half}_{pr}")
                    rdn = agp.tile([128, 4, 256], F32, tag=f"rdenA{pr}",
                                   name=f"rdenA{half}_{pr}")
                    # denominator loads first: the reciprocal chain is the
                    # critical path into the P4 matmuls
                    for lh in range(2):
                        h = 2 * pr + lh
                        dview = ao[bass.ds(gbv, 4), 65 * h + 64, :]
                        nc.sync.dma_start(
                            dn[lh * 64 : lh * 64 + 64],
                            bass.AP(tensor=dview.tensor, offset=dview.offset,
                                    ap=[[0, 64]] + dview.ap))
                    for lh in range(2):
                        h = 2 * pr + lh
                        av = ao[bass.ds(gbv, 4), 65 * h : 65 * h + 64, :]
                        nc.sync.dma_start(
                            a[lh * 64 : lh * 64 + 64],
                            av.rearrange("s p q -> p s q"))
                    tiles.append((a, dn, rdn))
                    agsb.append(a)
                # normalize split by output row-tile so the first projection
                # matmuls start after half the reciprocal work
                for qt in range(2):
                    for a, dn, rdn in tiles:
                        sl = (slice(None), slice(None),
                              slice(qt * 128, qt * 128 + 128))
                        nc.vector.reciprocal(rdn[sl], dn[sl])
                        nc.gpsimd.tensor_mul(a[sl], a[sl], rdn[sl])
                for qt in range(2):
                    ost = osb.tile([128, D], F32, tag="ost",
                                   name=f"ost{half}_{qt}")
                    for o0 in range(0, D, 512):
                        op = scps.tile([128, 512], F32, tag="sc",
                                       name=f"op{half}_{qt}_{o0}")
                        for pr in range(2):
                            for s in range(4):
                                nc.tensor.matmul(
                                    op[:],
                                    agsb[pr][:, s, qt * 128 : (qt + 1) * 128],
                                    wos[:, s * 2 + pr, o0 : o0 + 512],
                                    start=(pr == 0 and s == 0),
                                    stop=(pr == 1 and s == 3))
                        nc.vector.tensor_copy(ost[:, o0 : o0 + 512], op[:])
                    nc.sync.dma_start(
                        out[half * 256 + qt * 128
                            : half * 256 + (qt + 1) * 128, :],
                        ost[:])

            # ---------------- emission schedule ----------------
            # Phase 1: all four token-quarters (PE-bound on the qkv
            # projection; DVE/Pool run rope/stats; XBAR transposes trail
            # each quarter). Phase 2: half-0 attention, with its two
            # pair-AllGathers overlapping half-1 attention. Phase 3: half-1
            # attention with per-head AllGathers (short tail) and the
            # out-projections.
            xqs = []
            for qtr in range(4):
                xqt = persist.tile([128, 8, 512], BF16, name=f"xq{qtr}")
                nc.sync.dma_start(
                    xqt[:], xTr[:, :, qtr * 512 : qtr * 512 + 512])
                xqs.append(xqt)

            with (
                tc.tile_pool(name="p12sb", bufs=2) as p12sb,
                tc.tile_pool(name="p12ps", bufs=2, space="PSUM") as p12ps,
            ):
                for qtr in range(4):
                    emit_p12_body(qtr, p12sb, p12ps, xqs[qtr])
                    emit_p12_tps(qtr, nc.sync)

            with (
                tc.tile_pool(name="scps", bufs=3, space="PSUM") as scps,
                tc.tile_pool(name="atps", bufs=1, space="PSUM") as atps,
                tc.tile_pool(name="pbsb", bufs=3) as pbsb,
                tc.tile_pool(name="recp", bufs=2) as recp,
                tc.tile_pool(name="osb", bufs=2) as osb,
                tc.tile_pool(name="agp", bufs=2) as agp,
            ):
                for h in range(4):
                    emit_p3_head(0, h, scps, atps, pbsb, recp)
                    if h == 1:
                        nc.sync.dma_start(
                            wos[:],
                            wo[:].rearrange("(kc p) o -> p kc o", p=128))
                emit_a2a(0)
                for h in range(4):
                    emit_p3_head(1, h, scps, atps, pbsb, recp)
                emit_a2a(1)
                emit_p4(0, agp, osb, scps)
                emit_p4(1, agp, osb, scps)

    nc.compile()
    return nc


_PROG_CACHE = {}


def _get_program(key):
    if key not in _PROG_CACHE:
        _PROG_CACHE[key] = build_program(*key)
    return _PROG_CACHE[key]


def _host_inputs(x, W_qkv, b_qkv, W_out, b_out, q_gamma, k_gamma):
    import ml_dtypes
    mmnp = ml_dtypes.bfloat16
    x = np.asarray(x, dtype=np.float32)
    W_qkv = np.asarray(W_qkv, dtype=np.float32)
    b_qkv = np.asarray(b_qkv, dtype=np.float32)
    W_out = np.asarray(W_out, dtype=np.float32)
    q_gamma = np.asarray(q_gamma, dtype=np.float32)
    k_gamma = np.asarray(k_gamma, dtype=np.float32)

    apply_gamma = not (np.all(q_gamma == 1.0) and np.all(k_gamma == 1.0))
    qkv_bias = bool(np.any(b_qkv))

    # rope tables: pos = t*128 + p
    pos = np.arange(L, dtype=np.float64).reshape(NT, 128).T  # [128, NT]
    inv = 1.0 / (ROPE_THETA ** (np.arange(32, dtype=np.float64) / 32.0))
    ang = pos[:, :, None] * inv[None, None, :]               # [128, NT, 32]
    cosb = np.ascontiguousarray(np.broadcast_to(
        np.cos(ang)[:, :, None, :], (128, NT, 8, 32))).astype(mmnp)
    sinb = np.ascontiguousarray(np.broadcast_to(
        np.sin(ang)[:, :, None, :], (128, NT, 8, 32))).astype(mmnp)

    Wq = W_qkv[0 * D : 1 * D]
    Wk = W_qkv[1 * D : 2 * D]
    Wv = W_qkv[2 * D : 3 * D]
    WoT = np.ascontiguousarray(W_out.T)  # [d_in, d_out]

    in_maps = []
    for c in range(N_CORES):
        b = c // 4
        h0 = 4 * (c % 4)
        rows = slice(h0 * DH, (h0 + HPC) * DH)
        wq_c = np.ascontiguousarray(
            np.concatenate([Wq[rows], Wk[rows], Wv[rows]], axis=0).T)
        m = {
            "xT": np.ascontiguousarray(x[b].T).astype(mmnp),
            "wq": wq_c.astype(mmnp),
            "wo": WoT.astype(mmnp),
            "cosb": cosb,
            "sinb": sinb,
            "gb": np.array([[4 * (c // 4)]], dtype=np.uint32),
        }
        if apply_gamma:
            m["gam"] = np.ascontiguousarray(
                np.broadcast_to(q_gamma * k_gamma, (4, DH)).astype(np.float32))
        if qkv_bias:
            m["bqk"] = np.ascontiguousarray(np.concatenate(
                [b_qkv[0 * D : 1 * D][rows], b_qkv[1 * D : 2 * D][rows],
                 b_qkv[2 * D : 3 * D][rows]]))
        in_maps.append(m)

    key = (apply_gamma, qkv_bias)
    return key, in_maps


def _assemble(results, b_out):
    y = np.empty((B, L, D), dtype=np.float32)
    for c in range(N_CORES):
        b = c // 4
        r = c % 4
        o = results[c]["out"]
        for half in range(2):
            rows = slice(1024 * half + 256 * r, 1024 * half + 256 * r + 256)
            y[b, rows, :] = o[256 * half : 256 * half + 256]
    b_out = np.asarray(b_out, dtype=np.float32)
    if np.any(b_out):
        y += b_out
    return y


def _install_ntff_hook():
    """Register the axon NTFF profiling hook (the container's antenv stub
    lacks axon_hooks; replicate what trn_boot would have registered)."""
    import sys
    import types
    try:
        from antenv.axon_hooks import get_axon_ntff_profile_hook  # noqa: F401
        return
    except ImportError:
        pass
    try:
        from trn_agent_boot.trn_boot import _ntff_profile_via_ctypes
        hook = _ntff_profile_via_ctypes("/opt/axon/libaxon_pjrt.so")
    except Exception:
        hook = None
    import antenv
    mod = types.ModuleType("antenv.axon_hooks")
    mod.get_axon_ntff_profile_hook = lambda: hook
    mod.set_axon_ntff_profile_hook = lambda h: None
    antenv.axon_hooks = mod
    sys.modules["antenv.axon_hooks"] = mod


def kernel(x, W_qkv, b_qkv, W_out, b_out, q_gamma, k_gamma, _trace=False):
    from concourse.bass_utils import run_bass_kernel_spmd
    if _trace:
        _install_ntff_hook()

    key, in_maps = _host_inputs(x, W_qkv, b_qkv, W_out, b_out,
                                q_gamma, k_gamma)
    nc = _get_program(key)
    res = run_bass_kernel_spmd(nc, in_maps, core_ids=list(range(N_CORES)),
                               trace=_trace,
                               trace_cores=list(range(N_CORES)) if _trace else None)
    y = _assemble(res.results, b_out)
    if _trace:
        return y, res
    return y


# revision 33
# speedup vs baseline: 1.1593x; 1.1315x over previous
"""Block-causal sparse attention (QKNorm + RoPE) for Trainium2, 8 NeuronCores.

Sharding: batch*head parallel. 2 batches x 16 heads = 32 (b,h) pairs; core c
handles batch c//4, heads 4*(c%4) .. 4*(c%4)+4. The out-projection runs on
AllGather'd per-head attention within each batch's 4-core group; each core
writes 512 of the 2048 query rows.

v2 (from the 383us baseline):
  - Emission order overlaps the AllGathers with the other query-half's
    attention compute instead of stalling the pipeline on them (the baseline
    lost ~120us to engines idling around the 4 collectives).
  - q/k [token,feat] -> [feat,token] transposes run on the XBAR DMA engine
    (dma_start_transpose) instead of PE transpose + DVE copies; they are
    emitted deferred so they never head-of-line-block a compute queue.
  - k's RMS norm is folded into exp's per-partition scale (1/(8*rms_k) per
    key token): k-hat is never materialized, the normalize multiply covers
    only the q half, and exp needs no max-subtraction since |q-hat|=8 and
    the folded scale bounds |exponent| <= 8.
  - The denominator chain (repack/reciprocal/broadcast) moved off DVE: the
    reciprocal is a Pool-engine divide, the re-broadcast multiply runs on
    Pool, and the DMAs ride the SP hardware-DGE queue.
  - qkv PSUM->SBUF staging goes over DMA instead of the scalar engine (the
    scalar engine is saturated by softmax exp).
  - P2 statistics/rope are split across DVE and Pool; rope cos/sin tables
    are stored once [128, NT, 32] and broadcast via 0-stride views.

All matmuls are bf16 (fp8 was evaluated: every fp8 injection point alone
pushes rel-err past the 2e-2 gate; bf16 sits at ~6e-3). PSUM accumulation
and softmax statistics stay fp32.
"""

import numpy as np

import concourse.bass as bass
from concourse import bacc
import concourse.mybir as mybir
import concourse.tile as tile

F32 = mybir.dt.float32
BF16 = mybir.dt.bfloat16

B, L, D = 2, 2048, 1024
H, DH = 16, 64
NT = L // 128        # 16 token tiles
HPC = 4              # heads per core
N_CORES = 8
GROUPS = [[0, 1, 2, 3], [4, 5, 6, 7]]
ROPE_THETA = 10000.0
EPS = 1e-6

FQK = 512            # q+k feature columns per core
FV = 256
FTOT = 768


def build_program(apply_gamma=False, qkv_bias=False):
    nc = bacc.Bacc(num_devices=N_CORES)

    xT = nc.declare_dram_parameter("xT", [D, L], BF16, isOutput=False)
    gbp = nc.declare_dram_parameter("gb", [1, 1], mybir.dt.uint32, isOutput=False)
    wq = nc.declare_dram_parameter("wq", [D, FTOT], BF16, isOutput=False)
    wo = nc.declare_dram_parameter("wo", [D, D], BF16, isOutput=False)
    cosb = nc.declare_dram_parameter("cosb", [128, NT, 8, 32], BF16, isOutput=False)
    sinb = nc.declare_dram_parameter("sinb", [128, NT, 8, 32], BF16, isOutput=False)
    if apply_gamma:
        # combined per-element gamma (gq*gk), applied to the q half only
        gam = nc.declare_dram_parameter("gam", [4, DH], F32, isOutput=False)
    if qkv_bias:
        bqk = nc.declare_dram_parameter("bqk", [FTOT], F32, isOutput=False)
    # rows [256*half + r] = my shard of query rows [1024*half + 256*rank + r]
    out = nc.declare_dram_parameter("out", [L // 4, D], F32, isOutput=True)

    with tile.TileContext(nc) as tc:
        with (
            tc.tile_pool(name="singles", bufs=1) as singles,
            tc.tile_pool(name="persist", bufs=1) as persist,
            tc.tile_pool(name="dram", bufs=1, space="DRAM") as dram,
        ):
            # V with ones column appended: [tok%128, tile, head, 65]
            vsb = persist.tile([128, NT, HPC, DH + 1], BF16)
            nc.vector.memset(vsb[:, :, :, DH : DH + 1], 1.0)

            # transposed q,k: [feat-of-pair, pr, tok]; pr 0,1 = q pairs,
            # pr 2,3 = k pairs (matches the 3D XBAR transpose row split)
            QKT = persist.tile([128, 4, L], BF16)

            # stats: grp 0..3 = 1/rms_q; grp 4..7 = 1/(8*rms_k) (softmax
            # scale folded in)
            ssqp = persist.tile([128, NT, 8], F32)

            # sqrt bias tiles (eps for the q groups, 64*eps for k groups)
            epst = singles.tile([128, 1], F32)
            nc.gpsimd.memset(epst[:], EPS)
            epskt = singles.tile([128, 1], F32)
            nc.gpsimd.memset(epskt[:], DH * EPS)

            # rope tables, materialized host-side per-group ([0,8]-stride
            # broadcast DMAs explode into 64B descriptors; 0-stride SBUF
            # reads run ~3x slow on DVE)
            cost = singles.tile([128, NT, 8, 32], BF16, name="cos8")
            nc.sync.dma_start(cost[:], cosb[:])
            sint = singles.tile([128, NT, 8, 32], BF16, name="sin8")
            nc.sync.dma_start(sint[:], sinb[:])

            # dedicated prob tile for (half1, j0) with the block-causal
            # exclusion columns pre-zeroed once (no mid-flow memsets)
            pbm = persist.tile([128, 1024], BF16)
            nc.gpsimd.memset(pbm[:, 896:1024], 0.0)

            if apply_gamma:
                gamt = singles.tile([128, 4, DH], F32)
                nc.sync.dma_start(
                    gamt[:],
                    bass.AP(tensor=gam.tensor, offset=gam[:].offset,
                            ap=[[0, 128]] + gam[:].ap))
            if qkv_bias:
                bqkt = singles.tile([128, FTOT], F32)
                nc.sync.dma_start(
                    bqkt[:],
                    bass.AP(tensor=bqk.tensor, offset=bqk[:].offset,
                            ap=[[0, 128]] + bqk[:].ap))

            # qkv weights, split by K-chunk pairs
            wqt = []
            wqr = wq[:].rearrange("(kc p) f -> p kc f", p=128)
            for wi in range(4):
                wt = singles.tile([128, 2, FTOT], BF16, name=f"wqt{wi}")
                nc.sync.dma_start(wt[:], wqr[:, 2 * wi : 2 * wi + 2, :])
                wqt.append(wt)

            wos = persist.tile([128, 8, D], BF16)

            # group-base row of this core's batch within the 8-row A2A
            gbreg = nc.sync.alloc_register("gb_r")
            nc.sync.reg_load(gbreg, gbp[0:1, 0:1])
            gbv = nc.sync.snap(gbreg, donate=True, min_val=0, max_val=4)

            xTr = xT[:].rearrange("(kc p) l -> p kc l", p=128)
            qkrots = {}

            # ---------------- P1 + P2 (per token-quarter) ----------------
            def emit_p12_body(qtr, sbp, psp, xq):
                t0 = qtr * 4

                qkraw = sbp.tile([128, 4, 8, DH], F32, tag="qkraw",
                                 name=f"qkraw{qtr}")

                for t4 in range(4):
                    t = t0 + t4
                    qk_t = psp.tile([128, FQK], F32, tag="qk_ps",
                                    name=f"qk{qtr}_{t4}")
                    v_t = psp.tile([128, FV], F32, tag="v_ps",
                                   name=f"v{qtr}_{t4}")
                    qk_ps = qk_t[:]
                    v_ps = v_t[:]
                    for kc in range(8):
                        lhsT = xq[:, kc, t4 * 128 : t4 * 128 + 128]
                        wv = wqt[kc // 2][:, kc % 2]
                        nc.tensor.matmul(
                            qk_ps, lhsT, wv[:, 0:FQK],
                            start=(kc == 0), stop=(kc == 7))
                        nc.tensor.matmul(
                            v_ps, lhsT, wv[:, FQK:FTOT],
                            start=(kc == 0), stop=(kc == 7))
                    if qkv_bias:
                        nc.vector.tensor_add(
                            qkraw[:, t4],
                            qk_ps.rearrange("p (g d) -> p g d", d=DH),
                            bqkt[:, 0:FQK].rearrange("p (g d) -> p g d", d=DH))
                        nc.vector.tensor_add(
                            vsb[:, t, :, 0:DH],
                            v_ps.rearrange("p (g d) -> p g d", d=DH),
                            bqkt[:, FQK:FTOT].rearrange("p (g d) -> p g d", d=DH))
                    else:
                        # PSUM->SBUF staging on the scalar engine (idle
                        # during the projection phase)
                        nc.scalar.copy(
                            qkraw[:, t4],
                            qk_ps.rearrange("p (g d) -> p g d", d=DH))
                        nc.scalar.copy(
                            vsb[:, t, :, 0:DH],
                            v_ps.rearrange("p (g d) -> p g d", d=DH))

                # RMS statistics (square on the scalar engine: it is idle
                # during this phase, DVE is the pacer)
                sqt = sbp.tile([128, 4, 8, DH], BF16, tag="sqt",
                               name=f"sqt{qtr}")
                nc.scalar.activation(sqt[:], qkraw[:],
                                     mybir.ActivationFunctionType.Square)
                ssq = ssqp[:, t0 : t0 + 4, :]
                nc.vector.reduce_sum(ssq, sqt[:], axis=mybir.AxisListType.X)
                nc.scalar.activation(
                    ssqp[:, t0 : t0 + 4, 0:4], ssqp[:, t0 : t0 + 4, 0:4],
                    mybir.ActivationFunctionType.Sqrt,
                    bias=epst[:], scale=1.0 / DH)
                nc.scalar.activation(
                    ssqp[:, t0 : t0 + 4, 4:8], ssqp[:, t0 : t0 + 4, 4:8],
                    mybir.ActivationFunctionType.Sqrt,
                    bias=epskt[:], scale=1.0)
                nc.vector.reciprocal(ssq, ssq)

                # RoPE
                cqv = cost[:, t0 : t0 + 4, :, :]
                sqv = sint[:, t0 : t0 + 4, :, :]
                qkrot = sbp.tile([128, 4, 8, DH], BF16, tag="qkrot",
                                 name=f"qkrot{qtr}")
                qkrots[qtr] = qkrot
                q1 = qkraw[:, :, :, 0:32]
                q2 = qkraw[:, :, :, 32:64]
                mA = sbp.tile([128, 4, 8, 32], BF16, tag="mA", name=f"mA{qtr}")
                mB = sbp.tile([128, 4, 8, 32], BF16, tag="mB", name=f"mB{qtr}")
                nc.vector.tensor_mul(mA[:], q1, cqv)
                nc.vector.tensor_mul(mB[:], q2, sqv)
                nc.gpsimd.tensor_sub(qkrot[:, :, :, 0:32], mA[:], mB[:])
                mC = sbp.tile([128, 4, 8, 32], BF16, tag="mA", name=f"mC{qtr}")
                mD = sbp.tile([128, 4, 8, 32], BF16, tag="mB", name=f"mD{qtr}")
                nc.gpsimd.tensor_mul(mC[:], q2, cqv)
                nc.gpsimd.tensor_mul(mD[:], q1, sqv)
                nc.gpsimd.tensor_add(qkrot[:, :, :, 32:64], mC[:], mD[:])

                # apply 1/rms to the q half only (k's is folded into exp)
                rq = ssqp[:, t0 : t0 + 4, 0:4]
                rqv = bass.AP(tensor=rq.tensor, offset=rq.offset,
                              ap=rq.ap + [[0, DH]])
                nc.vector.tensor_mul(qkrot[:, :, 0:4, :],
                                     qkrot[:, :, 0:4, :], rqv)
                if apply_gamma:
                    gview = bass.AP(
                        tensor=gamt.tensor, offset=gamt[:].offset,
                        ap=[gamt[:].ap[0], [0, 4]] + gamt[:].ap[1:])
                    nc.vector.tensor_mul(qkrot[:, :, 0:4, :],
                                         qkrot[:, :, 0:4, :], gview)

            def emit_p12_tps(qtr, eng):
                # XBAR DMA transpose, 3D out: in [128 tok, 512 feat] ->
                # out[f, pr, tok] = in[tok, pr*128 + f]
                t0 = qtr * 4
                qkrot = qkrots[qtr]
                for t4 in range(4):
                    t = t0 + t4
                    eng.dma_start_transpose(
                        QKT[:, :, t * 128 : (t + 1) * 128],
                        qkrot[:, t4, :, :])

            # ---------------- P3: one head of one query half ----------------
            def emit_p3_head(half, h, scps, atps, pbsb, recp):
                qlo = half * 1024
                jmax = 8 if half == 0 else 16
                pb0 = (h % 2) * 64
                at = atps.tile([DH + 1, 1024], F32, tag="at",
                               name=f"at{half}_{h}")
                for j in range(jmax):
                    wlo = max(j * 128, qlo)     # global query col start
                    w = qlo + 1024 - wlo
                    sc = scps.tile([128, 1024], F32, tag="sc",
                                   name=f"sc{half}_{h}_{j}")
                    for c0 in range(0, w, 512):
                        cw = min(512, w - c0)
                        nc.tensor.matmul(
                            sc[:, c0 : c0 + cw],
                            QKT[pb0 : pb0 + 64, 2 + h // 2,
                                j * 128 : (j + 1) * 128],
                            QKT[pb0 : pb0 + 64, h // 2,
                                wlo + c0 : wlo + c0 + cw],
                            start=True, stop=True)
                    if half == 1 and j == 0:
                        # mask tile: cols 896.. pre-zeroed (last query frame
                        # can't see key frame 0); exp writes only 0..896
                        pb = pbm
                        wexp = 896
                    else:
                        pb = pbsb.tile([128, 1024], BF16, tag="pb",
                                       name=f"pb{half}_{h}_{j}")
                        wexp = w
                    # exp with k's 1/(8*rms) folded in as per-key scale
                    rk = ssqp[:, j, 4 + h : 5 + h]
                    nc.scalar.activation(
                        pb[:, 0:wexp], sc[:, 0:wexp],
                        mybir.ActivationFunctionType.Exp, scale=rk)
                    s_rel = wlo - qlo           # window start within half
                    for b0 in range(0, 1024, 512):
                        seg0 = max(s_rel, b0)
                        seg1 = b0 + 512
                        if seg0 >= seg1:
                            continue
                        nc.tensor.matmul(
                            at[:, seg0:seg1],
                            vsb[:, j, h, :],
                            pb[:, seg0 - s_rel : seg1 - s_rel],
                            start=(j == 0),
                            stop=(j == jmax - 1
                                  or (j + 1) * 128 >= qlo + seg1))
                # stage attn rows + the denominator row together, then
                # scatter into the AllToAll input (q-sharded per dest core);
                # normalization happens post-exchange on the P4 side
                st = recp.tile([DH + 1, 1024], BF16, tag="st65",
                               name=f"st{half}_{h}")
                nc.vector.tensor_copy(st[:], at[:])
                stv = st[:].rearrange("r (g c) -> r g c", g=4)
                for b in range(2):
                    dst = a2a_in[half][4 * b : 4 * b + 4,
                                       65 * h : 65 * h + 65, :]
                    nc.sync.dma_start(dst.rearrange("g r c -> r g c"), stv)

            # ------------- AllToAll exchange (one per half) -------------
            # 8-core mesh AllToAll (4-core groups are not mesh-supported).
            # in[g] = my 4 heads' (attn+den) rows for query columns
            # 256*(g%4); receivers ignore cross-batch blocks. out[s] = core
            # s's heads for my query shard; P4 reads rows [gb, gb+4).
            a2a_in = [dram.tile([8, 4 * 65, 256], BF16, name=f"a2ain{hf}")
                      for hf in range(2)]
            a2a_out = [dram.tile([8, 4 * 65, 256], BF16, name=f"a2aout{hf}")
                       for hf in range(2)]

            def emit_a2a(half):
                nc.gpsimd.collective_compute(
                    "AllToAll", mybir.AluOpType.bypass,
                    replica_groups=[list(range(8))],
                    ins=[a2a_in[half][:].opt()],
                    outs=[a2a_out[half][:].opt()])

            def emit_p4(half, agp, osb, scps):
                # a2a_out[half][s, 65*h + r, q'] = source core s's head h:
                # rows 0..63 unnormalized attn channels, row 64 the denom
                ao = a2a_out[half]
                agsb = []
                for pr in range(2):
                    a = agp.tile([128, 4, 256], BF16, tag=f"agsb{pr}",
                                 name=f"agsb{half}_{pr}")
                    dn = agp.tile([128, 4, 256], BF16, tag=f"denA{pr}",
                                  name=f"denA{half}_{pr}")
                    for lh in range(2):
                        h = 2 * pr + lh
                        av = ao[bass.ds(gbv, 4), 65 * h : 65 * h + 64, :]
                        nc.sync.dma_start(
                            a[lh * 64 : lh * 64 + 64],
                            av.rearrange("s p q -> p s q"))
                        dview = ao[bass.ds(gbv, 4), 65 * h + 64, :]
                        nc.sync.dma_start(
                            dn[lh * 64 : lh * 64 + 64],
                            bass.AP(tensor=dview.tensor, offset=dview.offset,
                                    ap=[[0, 64]] + dview.ap))
                    rdn = agp.tile([128, 4, 256], F32, tag=f"rdenA{pr}",
                                   name=f"rdenA{half}_{pr}")
                    nc.vector.reciprocal(rdn[:], dn[:])
                    nc.gpsimd.tensor_mul(a[:], a[:], rdn[:])
                    agsb.append(a)
                for qt in range(2):
                    ost = osb.tile([128, D], F32, tag="ost",
                                   name=f"ost{half}_{qt}")
                    for o0 in range(0, D, 512):
                        op = scps.tile([128, 512], F32, tag="sc",
                                       name=f"op{half}_{qt}_{o0}")
                        for pr in range(2):
                            for s in range(4):
                                nc.tensor.matmul(
                                    op[:],
                                    agsb[pr][:, s, qt * 128 : (qt + 1) * 128],
                                    wos[:, s * 2 + pr, o0 : o0 + 512],
                                    start=(pr == 0 and s == 0),
                                    stop=(pr == 1 and s == 3))
                        nc.vector.tensor_copy(ost[:, o0 : o0 + 512], op[:])
                    nc.sync.dma_start(
                        out[half * 256 + qt * 128
                            : half * 256 + (qt + 1) * 128, :],
                        ost[:])

            # ---------------- emission schedule ----------------
            # Phase 1: all four token-quarters (PE-bound on the qkv
            # projection; DVE/Pool run rope/stats; XBAR transposes trail
            # each quarter). Phase 2: half-0 attention, with its two
            # pair-AllGathers overlapping half-1 attention. Phase 3: half-1
            # attention with per-head AllGathers (short tail) and the
            # out-projections.
            xqs = []
            for qtr in range(4):
                xqt = persist.tile([128, 8, 512], BF16, name=f"xq{qtr}")
                nc.sync.dma_start(
                    xqt[:], xTr[:, :, qtr * 512 : qtr * 512 + 512])
                xqs.append(xqt)

            with (
                tc.tile_pool(name="p12sb", bufs=2) as p12sb,
                tc.tile_pool(name="p12ps", bufs=2, space="PSUM") as p12ps,
            ):
                for qtr in range(4):
                    emit_p12_body(qtr, p12sb, p12ps, xqs[qtr])
                    emit_p12_tps(qtr, nc.sync)

            with (
                tc.tile_pool(name="scps", bufs=3, space="PSUM") as scps,
                tc.tile_pool(name="atps", bufs=1, space="PSUM") as atps,
                tc.tile_pool(name="pbsb", bufs=3) as pbsb,
                tc.tile_pool(name="recp", bufs=2) as recp,
                tc.tile_pool(name="osb", bufs=2) as osb,
                tc.tile_pool(name="agp", bufs=2) as agp,
            ):
                for h in range(4):
                    emit_p3_head(0, h, scps, atps, pbsb, recp)
                    if h == 1:
                        nc.sync.dma_start(
                            wos[:],
                            wo[:].rearrange("(kc p) o -> p kc o", p=128))
                emit_a2a(0)
                for h in range(4):
                    emit_p3_head(1, h, scps, atps, pbsb, recp)
                emit_a2a(1)
                emit_p4(0, agp, osb, scps)
                emit_p4(1, agp, osb, scps)

    nc.compile()
    return nc


_PROG_CACHE = {}


def _get_program(key):
    if key not in _PROG_CACHE:
        _PROG_CACHE[key] = build_program(*key)
    return _PROG_CACHE[key]


def _host_inputs(x, W_qkv, b_qkv, W_out, b_out, q_gamma, k_gamma):
    import ml_dtypes
    mmnp = ml_dtypes.bfloat16
    x = np.asarray(x, dtype=np.float32)
    W_qkv = np.asarray(W_qkv, dtype=np.float32)
    b_qkv = np.asarray(b_qkv, dtype=np.float32)
    W_out = np.asarray(W_out, dtype=np.float32)
    q_gamma = np.asarray(q_gamma, dtype=np.float32)
    k_gamma = np.asarray(k_gamma, dtype=np.float32)

    apply_gamma = not (np.all(q_gamma == 1.0) and np.all(k_gamma == 1.0))
    qkv_bias = bool(np.any(b_qkv))

    # rope tables: pos = t*128 + p
    pos = np.arange(L, dtype=np.float64).reshape(NT, 128).T  # [128, NT]
    inv = 1.0 / (ROPE_THETA ** (np.arange(32, dtype=np.float64) / 32.0))
    ang = pos[:, :, None] * inv[None, None, :]               # [128, NT, 32]
    cosb = np.ascontiguousarray(np.broadcast_to(
        np.cos(ang)[:, :, None, :], (128, NT, 8, 32))).astype(mmnp)
    sinb = np.ascontiguousarray(np.broadcast_to(
        np.sin(ang)[:, :, None, :], (128, NT, 8, 32))).astype(mmnp)

    Wq = W_qkv[0 * D : 1 * D]
    Wk = W_qkv[1 * D : 2 * D]
    Wv = W_qkv[2 * D : 3 * D]
    WoT = np.ascontiguousarray(W_out.T)  # [d_in, d_out]

    in_maps = []
    for c in range(N_CORES):
        b = c // 4
        h0 = 4 * (c % 4)
        rows = slice(h0 * DH, (h0 + HPC) * DH)
        wq_c = np.ascontiguousarray(
            np.concatenate([Wq[rows], Wk[rows], Wv[rows]], axis=0).T)
        m = {
            "xT": np.ascontiguousarray(x[b].T).astype(mmnp),
            "wq": wq_c.astype(mmnp),
            "wo": WoT.astype(mmnp),
            "cosb": cosb,
            "sinb": sinb,
            "gb": np.array([[4 * (c // 4)]], dtype=np.uint32),
        }
        if apply_gamma:
            m["gam"] = np.ascontiguousarray(
                np.broadcast_to(q_gamma * k_gamma, (4, DH)).astype(np.float32))
        if qkv_bias:
            m["bqk"] = np.ascontiguousarray(np.concatenate(
                [b_qkv[0 * D : 1 * D][rows], b_qkv[1 * D : 2 * D][rows],
                 b_qkv[2 * D : 3 * D][rows]]))
        in_maps.append(m)

    key = (apply_gamma, qkv_bias)
    return key, in_maps


def _assemble(results, b_out):
    y = np.empty((B, L, D), dtype=np.float32)
    for c in range(N_CORES):
        b = c // 4
        r = c % 4
        o = results[c]["out"]
        for half in range(2):
            rows = slice(1024 * half + 256 * r, 1024 * half + 256 * r + 256)
            y[b, rows, :] = o[256 * half : 256 * half + 256]
    b_out = np.asarray(b_out, dtype=np.float32)
    if np.any(b_out):
        y += b_out
    return y


def _install_ntff_hook():
    """Register the axon NTFF profiling hook (the container's antenv stub
    lacks axon_hooks; replicate what trn_boot would have registered)."""
    import sys
    import types
    try:
        from antenv.axon_hooks import get_axon_ntff_profile_hook  # noqa: F401
        return
    except ImportError:
        pass
    try:
        from trn_agent_boot.trn_boot import _ntff_profile_via_ctypes
        hook = _ntff_profile_via_ctypes("/opt/axon/libaxon_pjrt.so")
    except Exception:
        hook = None
    import antenv
    mod = types.ModuleType("antenv.axon_hooks")
    mod.get_axon_ntff_profile_hook = lambda: hook
    mod.set_axon_ntff_profile_hook = lambda h: None
    antenv.axon_hooks = mod
    sys.modules["antenv.axon_hooks"] = mod


def kernel(x, W_qkv, b_qkv, W_out, b_out, q_gamma, k_gamma, _trace=False):
    from concourse.bass_utils import run_bass_kernel_spmd
    if _trace:
        _install_ntff_hook()

    key, in_maps = _host_inputs(x, W_qkv, b_qkv, W_out, b_out,
                                q_gamma, k_gamma)
    nc = _get_program(key)
    res = run_bass_kernel_spmd(nc, in_maps, core_ids=list(range(N_CORES)),
                               trace=_trace,
                               trace_cores=list(range(N_CORES)) if _trace else None)
    y = _assemble(res.results, b_out)
    if _trace:
        return y, res
    return y
